# revision 1
# baseline (speedup 1.0000x reference)
"""Multi-head self-attention block (B=4, N=2048, D=384, H=8, FF=1536) on 8 TRN2 cores.

Sharding: data-parallel over tokens. Core c handles batch b=c//2, query rows
[(c%2)*1024, (c%2+1)*1024). K/V are computed per-batch on each core (2x
replicated work, zero collectives). Everything on-device runs feature-major
(transposed); the host pre-transposes/pads inputs and unpads the output.

Head padding: each 48-dim head occupies a 64-row block laid out as
  rows 0-31  = head dims 0-31
  row  32    = ZERO (in Q/K/W1-input) -- reserved so the softmax denominator,
               which the P@V matmul drops into output row 32 via a
               ones-column in V's block, lands on a 32-aligned partition
               (the BIR verifier rejects non-32-aligned partition bases)
  rows 33-48 = head dims 32-47
  rows 49-63 = zero
Scores contract over rows 0-48 (the zero row contributes nothing). After
attention, ot is compacted 512->384 rows by partition-moving SBUF->SBUF DMAs
so both FFN matmuls run over compact (unpadded) dimensions.

Hardware/compiler quirks this code works around:
  * fp32r matmul inputs must come from instructions whose output dtype is
    float32r ("rounded to FP32r" verifier rule); f32r memsets are invalid ISA
    (constants are memset f32 + DVE-copied);
  * tile_position with a nonzero column is invalid ISA in this neuronxcc, so
    both heads' P@V accumulate at partitions 0-63 of separate PSUM tiles and
    head B is partition-shifted 0->64 by an (aligned) DVE copy at the end;
  * the broadcast of the softmax denominator across partitions is a K=1
    ones-outer-product matmul (the gpsimd partition_broadcast ucode does not
    compile); TRN2 allows one sync-wait per instruction -- Bacc's
    generate_event_semaphores pass splits the rest.
"""

import math
import numpy as np

B, N, D, H, DH, DFF = 4, 2048, 384, 8, 48, 1536
PH = 64            # padded per-head dim
DP = H * PH        # 512 padded model dim
ROWS = 1024        # query rows per core
KD = D // 128      # 3 k-tiles over model dim
TQ = DP // 128     # 4 tiles over padded dim (= head pairs)
NJ = N // 128      # 16 key tiles
IC = ROWS // 512   # 2 i-chunks
NF = DFF // 128    # 12 ffn tiles
KH = DH + 1        # 49: contraction rows per head (incl the zero row 32)
DEN = 32           # block row where the denominator lands
SCALE = 1.0 / math.sqrt(D)

# position of head dim e inside its 64-row block (skips row 32)
PERM = np.array([e if e < DEN else e + 1 for e in range(DH)])

# DMA segments to compact padded ot [512 rows] -> otc [384 rows]:
# (src_tile, src_row, dst_tile, dst_row, nrows)
def _compact_segs():
    segs = []
    for h in range(H):
        for s_lo, s_hi, d_lo in ((0, DEN, DH * h), (DEN + 1, KH, DH * h + DEN)):
            off = 0
            while off < s_hi - s_lo:
                d = d_lo + off
                n = min(s_hi - s_lo - off, 128 - (d % 128))
                segs.append((h // 2, 64 * (h % 2) + s_lo + off, d // 128, d % 128, n))
                off += n
    return segs

CSEGS = _compact_segs()

_CACHE = {}


def _build():
    from contextlib import ExitStack
    import concourse.bass as bass
    import concourse.bacc as bacc
    import concourse.tile as tile
    import concourse.mybir as mybir

    F32 = mybir.dt.float32
    F32R = mybir.dt.float32r
    F16 = mybir.dt.float16
    AF = mybir.ActivationFunctionType
    ts = bass.ts

    nc = bacc.Bacc(trn_type="TRN2", target_bir_lowering=False, debug=False)

    def din(name, shape, dt=F32):
        return nc.dram_tensor(name, shape, dt, kind="ExternalInput").ap()

    xT = din("xT", [D, ROWS])
    yT = din("yT", [D, N])
    wqT = din("wqT", [D, DP])
    wkT = din("wkT", [D, DP])
    wvT = din("wvT", [D, D])
    w1T = din("w1T", [D, DFF])
    w2T = din("w2T", [DFF, D], F16)
    o = nc.dram_tensor("o", [D, ROWS], F32, kind="ExternalOutput").ap()

    with tile.TileContext(nc) as tc, ExitStack() as ctx:
        sb = ctx.enter_context(tc.tile_pool(name="sb", bufs=1))
        ps = ctx.enter_context(tc.tile_pool(name="ps", bufs=1, space="PSUM"))

        def load(dst, dram_ap, width, dt=F32R):
            if dt is F32R:
                nc.sync.dma_start(out=dst[:], in_=dram_ap.bitcast(F32R))
            else:
                nc.sync.dma_start(out=dst[:], in_=dram_ap)

        # ---- input loads (DMA directly into f32r-typed tiles) ----
        xt = [sb.tile([128, ROWS], F32R, tag="xq", bufs=7, name=f"xt{k}") for k in range(KD)]
        wq = [sb.tile([128, DP], F32R, tag="wqk", bufs=6, name=f"wq{k}") for k in range(KD)]
        yt = [sb.tile([128, N], F32R, tag="big", bufs=7, name=f"yt{k}") for k in range(KD)]
        wk = [sb.tile([128, DP], F32R, tag="wqk", bufs=6, name=f"wk{k}") for k in range(KD)]
        wv = [sb.tile([128, D], F32R, tag="wv", bufs=3, name=f"wv{k}") for k in range(KD)]
        for k in range(KD):
            load(xt[k], xT[ts(k, 128), :], ROWS)
            load(wq[k], wqT[ts(k, 128), :], DP)
        # first column-chunk of y plus the K/V weights lets the K projection,
        # first scores and first V tiles start ~20us earlier
        for k in range(KD):
            nc.sync.dma_start(out=yt[k][:, 0:512], in_=yT[ts(k, 128), 0:512].bitcast(F32R))
            load(wk[k], wkT[ts(k, 128), :], DP)
            load(wv[k], wvT[ts(k, 128), :], D)
        for n in range(1, N // 512):
            for k in range(KD):
                nc.sync.dma_start(out=yt[k][:, ts(n, 512)],
                                  in_=yT[ts(k, 128), ts(n, 512)].bitcast(F32R))

        # ---- projections (pair-0 prerequisites emitted first) ----
        qt = [sb.tile([128, ROWS], F32R, tag="xq", bufs=7, name=f"qt{t}") for t in range(TQ)]
        kt = [sb.tile([128, N], F32R, tag="big", bufs=7, name=f"kt{t}") for t in range(TQ)]

        def qproj(t):
            for c in range(IC):
                p = ps.tile([128, 512], F32, tag="pv", bufs=4, name=f"psq{t}_{c}")
                for k in range(KD):
                    nc.tensor.matmul(
                        p[:], wq[k][:, ts(t, 128)], xt[k][:, ts(c, 512)],
                        start=(k == 0), stop=(k == KD - 1))
                nc.vector.tensor_copy(qt[t][:, ts(c, 512)], p[:])

        def kproj(t, n):
            p = ps.tile([128, 512], F32, tag="pv", bufs=4, name=f"psk{t}_{n}")
            for k in range(KD):
                nc.tensor.matmul(
                    p[:], wk[k][:, ts(t, 128)], yt[k][:, ts(n, 512)],
                    start=(k == 0), stop=(k == KD - 1))
            nc.vector.tensor_copy(kt[t][:, ts(n, 512)], p[:])

        # constants: memset f32 then DVE-copy to f32r (f32r memset is invalid ISA)
        kf = sb.tile([128, 704], F32, tag="kf", bufs=1, name="kf")
        nc.vector.memset(kf[:, 0:64], 1.0)
        nc.vector.memset(kf[:, 64:704], 0.0)
        one64 = sb.tile([128, PH], F32R, tag="one64", bufs=1, name="one64")
        nc.vector.tensor_copy(one64[:], kf[:, 0:PH])

        # V row-major, augmented: vaug[j] = [128, 8*64]; per head block:
        # cols 0-31 = V dims 0-31, col 32 = 1.0 (denominator), cols 33-48 =
        # V dims 32-47, cols 49-63 = 0
        vaug = [sb.tile([128, DP], F32R, tag="v512", bufs=16, name=f"va{j}") for j in range(NJ)]

        def vproj(j):
            p = ps.tile([128, 512], F32, tag="pv", bufs=4, name=f"psv{j}")
            for k in range(KD):
                nc.tensor.matmul(
                    p[:, 0:D], yt[k][:, ts(j, 128)], wv[k][:],
                    start=(k == 0), stop=(k == KD - 1))
            va3 = vaug[j][:].rearrange("p (h e) -> p h e", h=H)
            ps3 = p[:, 0:D].rearrange("p (h e) -> p h e", h=H)
            nc.vector.tensor_copy(va3[:, :, 0:DEN], ps3[:, :, 0:DEN])
            nc.vector.tensor_copy(va3[:, :, DEN + 1:KH], ps3[:, :, DEN:DH])
            nc.vector.tensor_copy(va3[:, :, DEN:DEN + 1],
                                  kf[:, 0:H].rearrange("p (h e) -> p h e", h=H))
            nc.vector.tensor_copy(va3[:, :, KH:PH],
                                  kf[:, 576:576 + H * (PH - KH)].rearrange("p (h e) -> p h e", h=H))


        # ---- attention, one head pair (= one qt/kt tile) at a time ----
        otc = [sb.tile([128, ROWS], F32R, tag="otc", bufs=3, name=f"otc{m}") for m in range(KD)]

        def normalize_dve(t, pv):
            # all-DVE variant for the LAST pair: higher DVE cost but shortest
            # latency chain (no PE/DVE ping-pong) -- this pair's normalize is
            # exposed at the attention->FFN transition, not hidden
            ot = sb.tile([128, ROWS], F32R, tag="ot", bufs=2, name=f"otd{t}")
            for c in range(IC):
                for ab in range(2):
                    rr = sb.tile([128, 512], F32, tag="nrm", bufs=6, name=f"dr{t}_{ab}_{c}")
                    nc.vector.reciprocal(rr[DEN:DEN + 1, :], pv[ab][c][DEN:DEN + 1, :])
                    nc.vector.tensor_copy(rr[0:1, :], rr[DEN:DEN + 1, :])
                    rbt = sb.tile([128, 512], F32, tag="nrm", bufs=6, name=f"db{t}_{ab}_{c}")
                    nc.vector.stream_shuffle(rbt[0:64, :], rr[0:64, :], [0] * 32)
                    nc.vector.tensor_mul(rbt[0:64, :], pv[ab][c][0:64, :], rbt[0:64, :])
                    if ab == 0:
                        nc.vector.tensor_add(ot[0:64, ts(c, 512)], rbt[0:64, :],
                                             qt[t][0:64, ts(c, 512)])
                        for st_, sr, dt_, dr, nr in CSEGS:
                            if st_ == t and sr < 64:
                                nc.sync.dma_start(out=otc[dt_][dr:dr + nr, ts(c, 512)],
                                                  in_=ot[sr:sr + nr, ts(c, 512)])
                    else:
                        rbB2 = sb.tile([128, 512], F32, tag="nrm", bufs=6, name=f"db2{t}_{c}")
                        nc.vector.tensor_copy(rbB2[64:128, :], rbt[0:64, :])
                        nc.vector.tensor_add(ot[64:128, ts(c, 512)], rbB2[64:128, :],
                                             qt[t][64:128, ts(c, 512)])
                        for st_, sr, dt_, dr, nr in CSEGS:
                            if st_ == t and sr >= 64:
                                nc.sync.dma_start(out=otc[dt_][dr:dr + nr, ts(c, 512)],
                                                  in_=ot[sr:sr + nr, ts(c, 512)])

        def normalize(t, pv):
            ot = sb.tile([128, ROWS], F32R, tag="ot", bufs=2, name=f"ot{t}")
            # normalize by the denominator (row 32 of each pv tile) + residual;
            # the broadcast along partitions is a K=1 ones-outer-product
            # matmul. Phase-major across the two chunks so the PE/DVE
            # ping-pong of one chain hides under the other.
            rtA, rtB, rbA, rbB = {}, {}, {}, {}
            with nc.allow_low_precision(reason="f32r reciprocal for bcast"):
                for c in range(IC):
                    rtA[c] = sb.tile([128, 512], F32R, tag="nrm", bufs=6, name=f"rtA{t}_{c}")
                    rtB[c] = sb.tile([128, 512], F32R, tag="nrm", bufs=6, name=f"rtB{t}_{c}")
                    nc.vector.reciprocal(rtA[c][DEN:DEN + 1, :], pv[0][c][DEN:DEN + 1, :])
                    nc.vector.reciprocal(rtB[c][DEN:DEN + 1, :], pv[1][c][DEN:DEN + 1, :])
            rbp = ps.tile([128, 1024], F32, tag="st", bufs=2, name=f"rbp{t}_0")
            rbp2 = ps.tile([128, 1024], F32, tag="st", bufs=2, name=f"rbp{t}_1")
            for c, rp in ((0, rbp), (1, rbp2)):
                nc.tensor.matmul(rp[0:64, 0:512], one64[DEN:DEN + 1, :],
                                 rtA[c][DEN:DEN + 1, :],
                                 start=True, stop=True, tile_position=(DEN, 0))
                nc.tensor.matmul(rp[0:64, 512:1024], one64[DEN:DEN + 1, :],
                                 rtB[c][DEN:DEN + 1, :],
                                 start=True, stop=True, tile_position=(DEN, 0))
            for c, rp in ((0, rbp), (1, rbp2)):
                rbA[c] = sb.tile([128, 512], F32, tag="nrm", bufs=6, name=f"rbA{t}_{c}")
                rbB[c] = sb.tile([128, 512], F32, tag="nrm", bufs=6, name=f"rbB{t}_{c}")
                nc.vector.tensor_copy(rbA[c][0:64, :], rp[0:64, 0:512])
                nc.vector.tensor_copy(rbB[c][0:64, :], rp[0:64, 512:1024])
            for c in range(IC):
                nc.vector.tensor_mul(rbA[c][0:64, :], pv[0][c][0:64, :], rbA[c][0:64, :])
                nc.vector.tensor_mul(rbB[c][0:64, :], pv[1][c][0:64, :], rbB[c][0:64, :])
            for c in range(IC):
                nc.vector.tensor_add(ot[0:64, ts(c, 512)], rbA[c][0:64, :],
                                     qt[t][0:64, ts(c, 512)])
                # compact the finished A-half chunk into otc right away
                # (DMA moves partitions freely; overlaps the B-half chain)
                for st_, sr, dt_, dr, nr in CSEGS:
                    if st_ == t and sr < 64:
                        nc.sync.dma_start(out=otc[dt_][dr:dr + nr, ts(c, 512)],
                                          in_=ot[sr:sr + nr, ts(c, 512)])
                rbB2 = sb.tile([128, 512], F32, tag="nrm", bufs=6, name=f"rbB2{t}_{c}")
                nc.vector.tensor_copy(rbB2[64:128, :], rbB[c][0:64, :])  # aligned shift
                nc.vector.tensor_add(ot[64:128, ts(c, 512)], rbB2[64:128, :],
                                     qt[t][64:128, ts(c, 512)])
                for st_, sr, dt_, dr, nr in CSEGS:
                    if st_ == t and sr >= 64:
                        nc.sync.dma_start(out=otc[dt_][dr:dr + nr, ts(c, 512)],
                                          in_=ot[sr:sr + nr, ts(c, 512)])

        kproj(0, 0)
        qproj(0)
        for j in range(4):
            vproj(j)
        for n in range(1, N // 512):
            kproj(0, n)
        for t in range(1, TQ):
            qproj(t)
            for n in range(N // 512):
                kproj(t, n)
        for j in range(4, NJ):
            vproj(j)

        pending = None
        for t in range(TQ):
            # both heads' P@V accumulate at partitions 0-63 of separate PSUM
            # tiles (tile_position col != 0 is invalid ISA in this compiler)
            pv = [[ps.tile([128, 512], F32, tag="pv", bufs=4, name=f"pspv{t}_{ab}_{c}")
                   for c in range(IC)] for ab in range(2)]  # [headAB][chunk]
            for j in range(NJ):
                stA = ps.tile([128, 1024], F32, tag="st", bufs=2, name=f"stA{t}_{j}")
                stB = ps.tile([128, 1024], F32, tag="st", bufs=2, name=f"stB{t}_{j}")
                for c in range(IC):
                    nc.tensor.matmul(
                        stA[:, ts(c, 512)],
                        kt[t][0:KH, ts(j, 128)], qt[t][0:KH, ts(c, 512)],
                        start=True, stop=True, tile_position=(0, 0))
                    nc.tensor.matmul(
                        stB[:, ts(c, 512)],
                        kt[t][64:64 + KH, ts(j, 128)], qt[t][64:64 + KH, ts(c, 512)],
                        start=True, stop=True, tile_position=(64, 0))
                peA = sb.tile([128, 1024], F32R, tag="pt", bufs=4, name=f"peA{t}_{j}")
                peB = sb.tile([128, 1024], F32R, tag="pt", bufs=4, name=f"peB{t}_{j}")
                nc.scalar.activation(peA[:], stA[:], AF.Exp, scale=SCALE)
                nc.scalar.activation(peB[:], stB[:], AF.Exp, scale=SCALE)
                if j == 0 and pending is not None:
                    # previous pair's normalize first (frees the pv slots)
                    normalize(*pending)
                    pending = None
                for c in range(IC):
                    nc.tensor.matmul(
                        pv[0][c][0:PH, :],
                        vaug[j][:, ts(2 * t, PH)], peA[:, ts(c, 512)],
                        start=(j == 0), stop=(j == NJ - 1), tile_position=(0, 0))
                    nc.tensor.matmul(
                        pv[1][c][0:PH, :],
                        vaug[j][:, ts(2 * t + 1, PH)], peB[:, ts(c, 512)],
                        start=(j == 0), stop=(j == NJ - 1), tile_position=(0, 0))
            pending = (t, pv)
        normalize_dve(*pending)

        # ---- FFN (feature-major): ot -> gelu(W1@ot) -> W2@hid + ot ----
        # w2 copies BEFORE w1 copies: the first FFN1 matmul's DVE wait then
        # covers the w2 copies too, so FFN2 matmuls only wait on ACT (gelu)
        w2 = [sb.tile([128, D], F16, tag="v512", bufs=16, name=f"w2_{f}") for f in range(NF)]
        for f in range(NF):
            load(w2[f], w2T[ts(f, 128), :], D, dt=F16)
        w1 = [sb.tile([128, DFF], F32R, tag="w1", bufs=3, name=f"w1_{k}") for k in range(KD)]
        for k in range(KD):
            load(w1[k], w1T[ts(k, 128), :], DFF)

        osb = [sb.tile([128, ROWS], F32, tag="xq", bufs=7, name=f"osb{m}") for m in range(KD)]
        for c in range(IC):
            po = [ps.tile([128, 512], F32, tag="pv", bufs=4, name=f"po{c}_{m}")
                  for m in range(KD)]
            for g in range(NF // 2):
                sg = ps.tile([128, 1024], F32, tag="st", bufs=2, name=f"sg{c}_{g}")
                for fi in range(2):
                    f = g * 2 + fi
                    for k in range(KD):
                        nc.tensor.matmul(
                            sg[:, ts(fi, 512)],
                            w1[k][:, ts(f, 128)], otc[k][:, ts(c, 512)],
                            start=(k == 0), stop=(k == KD - 1))
                hf = sb.tile([128, 1024], F16, tag="hid", bufs=3, name=f"hf{c}_{g}")
                nc.scalar.activation(hf[:], sg[:], AF.Gelu)
                for m in range(KD):
                    for fi in range(2):
                        nc.tensor.matmul(
                            po[m][:], w2[g * 2 + fi][:, ts(m, 128)],
                            hf[:, fi * 512:(fi + 1) * 512],
                            start=(g == 0 and fi == 0), stop=(g == NF // 2 - 1 and fi == 1))
            for m in range(KD):
                nc.vector.tensor_add(osb[m][:, ts(c, 512)], po[m][:], otc[m][:, ts(c, 512)])
                nc.sync.dma_start(out=o[ts(m, 128), c * 512:(c + 1) * 512],
                                  in_=osb[m][:, ts(c, 512)])

    nc.compile()
    return nc


def _prep_weights(Wq, Wk, Wv, W1, W2):
    def pad_rows(w):  # [384, X] -> [512, X]; head dims at PERM rows per block
        out = np.zeros((DP,) + w.shape[1:], dtype=w.dtype)
        out.reshape(H, PH, -1)[:, PERM] = w.reshape(H, DH, -1)
        return out

    wqT = np.ascontiguousarray(pad_rows(Wq).T)            # [384, 512]
    wkT = np.ascontiguousarray(pad_rows(Wk).T)            # [384, 512]
    wvT = np.ascontiguousarray(Wv.T)                      # [384, 384]
    w1T = np.ascontiguousarray(W1.T)                      # [384, 1536] compact
    w2T = np.ascontiguousarray(W2.T).astype(np.float16)   # [1536, 384] compact
    return wqT, wkT, wvT, w1T, w2T


def _run(in_maps, trace=False):
    from concourse.bass_utils import run_bass_kernel_spmd

    if "nc" not in _CACHE:
        _CACHE["nc"] = _build()
    try:
        return run_bass_kernel_spmd(_CACHE["nc"], in_maps, list(range(8)), trace=trace)
    except Exception:
        # one retry: absorbs transient device wedges (NRT_EXEC_UNIT_* from a
        # previous interrupted run on the shared tunneled devices). Once PJRT
        # marks a device unrecoverable the client is poisoned, so drop the
        # cached backends to force a fresh client before retrying.
        import time as _time
        last = None
        for delay in (10.0, 30.0):
            try:
                import jax
                import jax._src.xla_bridge as _xb
                jax.clear_caches()
                with _xb._backend_lock:
                    _xb._backends.clear()
                    _xb._backend_errors.clear()
            except Exception:
                pass
            _time.sleep(delay)
            try:
                return run_bass_kernel_spmd(_CACHE["nc"], in_maps,
                                            list(range(8)), trace=trace)
            except Exception as e:  # noqa
                last = e
        raise last


def _make_in_maps(x, y, Wq, Wk, Wv, W1, W2):
    x = np.asarray(x, dtype=np.float32)
    y = np.asarray(y, dtype=np.float32)
    wqT, wkT, wvT, w1T, w2T = _prep_weights(
        np.asarray(Wq, np.float32), np.asarray(Wk, np.float32),
        np.asarray(Wv, np.float32), np.asarray(W1, np.float32),
        np.asarray(W2, np.float32))
    in_maps = []
    for c in range(8):
        b, half = c // 2, c % 2
        xs = x[b, half * ROWS:(half + 1) * ROWS]  # [1024, 384]
        in_maps.append({
            "xT": np.ascontiguousarray(xs.T),
            "yT": np.ascontiguousarray(y[b].T),
            "wqT": wqT, "wkT": wkT, "wvT": wvT, "w1T": w1T, "w2T": w2T,
        })
    return in_maps


def _unshard(results):
    out = np.empty((B, N, D), np.float32)
    for c in range(8):
        oc = results[c]["o"]  # [384, 1024] compact feature-major
        out[c // 2, (c % 2) * ROWS:(c % 2 + 1) * ROWS, :] = oc.T
    return out


def kernel(x, y, Wq, Wk, Wv, W1, W2):
    res = _run(_make_in_maps(x, y, Wq, Wk, Wv, W1, W2))
    return _unshard(res.results)


def profile(x, y, Wq, Wk, Wv, W1, W2):
    """Run with NTFF tracing; returns exec_time_ns (or None)."""
    import concourse.bass_utils as bu
    orig = bu.upload_artifacts
    bu.upload_artifacts = lambda tmpdir: f"file://{tmpdir}"
    try:
        res = _run(_make_in_maps(x, y, Wq, Wk, Wv, W1, W2), trace=True)
    finally:
        bu.upload_artifacts = orig
    return res.exec_time_ns



# revision 2
# speedup vs baseline: 1.1473x; 1.1473x over previous
"""Multi-head self-attention block (B=4, N=2048, D=384, H=8, FF=1536) on 8 TRN2 cores.

Sharding: data-parallel over tokens. Core c handles batch b=c//2, query rows
[(c%2)*1024, (c%2+1)*1024). K/V are computed per-batch on each core (2x
replicated work, zero collectives). Everything runs fp16 on the PE inputs
(f32 PSUM accumulation); the host pre-casts/pads inputs and unpads the output.

Head padding: each 48-dim head occupies a 64-slot block:
  slots 0-47 = head dims, slot 48 = softmax-denominator slot, 49-63 = junk.
Q/K are feature-major [512pad, n] with that row layout (wq/wk host-padded with
zero rows so the pad rows are zero). V is row-major "augmented": vaug[j] =
[128 keys, 8*64] with per-head block cols [V dims 0-47 | 1.0 | junk]; the ones
column makes the P@V matmul drop the softmax denominator into output col 48.

Attention datapath per head pair t (heads 2t, 2t+1):
  scores  S[j-tile, q] = K^T Q     (PSUM f32, keys on partitions)
  exp     ACT Exp for most tiles; a subset runs on DVE via the Schraudolph
          bit-trick (out_i16 = s*A16 + B16, bitcast to fp16) to offload the
          ACT engine, which is otherwise the bottleneck.
  P@V     TRANSPOSED: out[q, v] = sum_j P[j,q] V[j,v] -- queries on output
          partitions (full 128-wide PE use; 49-wide moving dim). 8 i-tile
          accumulators per head packed at 64-col offsets into one PSUM bank,
          zeroed by DVE memset and accumulated with start=False matmuls.
  norm    denominator is per-partition (col 48) -> DVE reciprocal +
          tensor_scalar multiply into o_r [128 q, 128] fp16 (A cols 0-48,
          B cols 64-112).
  back    one PE transpose per (t, i) -> [128 v, 128 q] fp16 in PSUM, then
          one DVE scalar_tensor_tensor adds the Q residual while copying to
          the padded feature-major ot_p.
ot_p is compacted 512->384 rows by 10 partition-moving SBUF->SBUF DMAs, then
the FFN (fp16 weights, f32 PSUM) runs over compact dims with a fused final
residual add.

PSUM budget (8 banks): st 2x[128,1024] (4) + acc 2x[128,512] (2) +
pj 2x[128,512] (2). pj serves projections, transposes (bitcast fp16 view),
and is free for FFN; acc serves attention accumulators and FFN2 accumulators.
"""

import math
import numpy as np

B, N, D, H, DH, DFF = 4, 2048, 384, 8, 48, 1536
PH = 64            # padded per-head block
DP = H * PH        # 512 padded model dim
ROWS = 1024        # query rows per core
KD = D // 128      # 3 k-tiles over model dim
TQ = DP // 128     # 4 tiles over padded dim (= head pairs)
NJ = N // 128      # 16 key tiles
NI = ROWS // 128   # 8 query i-tiles
NF = DFF // 128    # 12 ffn tiles
KH = DH + 1        # 49 cols per head block incl denominator col
SCALE = 1.0 / math.sqrt(D)

# Schraudolph fp16 exp: bitcast_f16(int16(s*A16 + B16)) ~= exp(s*SCALE)
A16 = SCALE * 1024.0 / math.log(2.0)
B16 = 15.0 * 1024.0 - 60.0

# exp tiles routed to DVE instead of ACT: (head_in_pair, j) pairs
DVE_EXP = {(1, j) for j in range(0, NJ, 2)}


# DMA segments to compact padded ot_p [512 rows] -> otc [384 rows]:
# (src_tile, src_row, dst_tile, dst_row, nrows)
def _compact_segs():
    segs = []
    for h in range(H):
        s_lo, d, left, off = 64 * (h % 2), DH * h, DH, 0
        while left:
            n = min(left, 128 - ((d + off) % 128))
            segs.append((h // 2, s_lo + off, (d + off) // 128, (d + off) % 128, n))
            off += n
            left -= n
    return segs


CSEGS = _compact_segs()

_CACHE = {}


def _build():
    from contextlib import ExitStack
    import concourse.bass as bass
    import concourse.bacc as bacc
    import concourse.tile as tile
    import concourse.mybir as mybir

    F32 = mybir.dt.float32
    F16 = mybir.dt.float16
    I16 = mybir.dt.int16
    AF = mybir.ActivationFunctionType
    ALU = mybir.AluOpType
    ts = bass.ts

    nc = bacc.Bacc(trn_type="TRN2", target_bir_lowering=False, debug=False)

    def din(name, shape, dt=F16):
        return nc.dram_tensor(name, shape, dt, kind="ExternalInput").ap()

    xT = din("xT", [D, ROWS])
    yT = din("yT", [D, N])
    wqT = din("wqT", [D, DP])
    wkT = din("wkT", [D, DP])
    wvT = din("wvT", [D, D])
    w1T = din("w1T", [D, DFF])
    w2T = din("w2T", [DFF, D])
    idT = din("idT", [128, 128])
    o = nc.dram_tensor("o", [D, ROWS], F32, kind="ExternalOutput").ap()

    with tile.TileContext(nc) as tc, ExitStack() as ctx:
        sb = ctx.enter_context(tc.tile_pool(name="sb", bufs=1))
        ps = ctx.enter_context(tc.tile_pool(name="ps", bufs=1, space="PSUM"))

        # ---- persistent SBUF tiles ----
        xt = [sb.tile([128, ROWS], F16, tag="xt", bufs=3, name=f"xt{k}") for k in range(KD)]
        yt = [sb.tile([128, N], F16, tag="yt", bufs=3, name=f"yt{k}") for k in range(KD)]
        wq = [sb.tile([128, DP], F16, tag="wqk", bufs=6, name=f"wq{k}") for k in range(KD)]
        wk = [sb.tile([128, DP], F16, tag="wqk", bufs=6, name=f"wk{k}") for k in range(KD)]
        wv = [sb.tile([128, D], F16, tag="wv", bufs=3, name=f"wv{k}") for k in range(KD)]
        qt = [sb.tile([128, ROWS], F16, tag="qt", bufs=4, name=f"qt{t}") for t in range(TQ)]
        kt = [sb.tile([128, N], F16, tag="kt", bufs=4, name=f"kt{t}") for t in range(TQ)]
        vaug = [sb.tile([128, DP], F16, tag="va", bufs=16, name=f"va{j}") for j in range(NJ)]
        ident = sb.tile([128, 128], F16, tag="id", bufs=1, name="ident")
        ot_p = [sb.tile([128, ROWS], F16, tag="otp", bufs=4, name=f"otp{t}") for t in range(TQ)]
        otc = [sb.tile([128, ROWS], F16, tag="otc", bufs=3, name=f"otc{m}") for m in range(KD)]
        w1 = [sb.tile([128, DFF], F16, tag="w1", bufs=3, name=f"w1_{k}") for k in range(KD)]
        w2 = [sb.tile([128, D], F16, tag="w2", bufs=12, name=f"w2_{f}") for f in range(NF)]

        # ---- input loads ----
        for k in range(KD):
            nc.sync.dma_start(out=xt[k][:], in_=xT[ts(k, 128), :])
            nc.sync.dma_start(out=wq[k][:], in_=wqT[ts(k, 128), :])
        for k in range(KD):
            nc.sync.dma_start(out=yt[k][:, 0:1024], in_=yT[ts(k, 128), 0:1024])
            nc.sync.dma_start(out=wk[k][:], in_=wkT[ts(k, 128), :])
            nc.sync.dma_start(out=wv[k][:], in_=wvT[ts(k, 128), :])
        nc.sync.dma_start(out=ident[:], in_=idT[:, :])
        for k in range(KD):
            nc.sync.dma_start(out=yt[k][:, 1024:2048], in_=yT[ts(k, 128), 1024:2048])

        def load_ffn_weights():
            for f in range(NF):
                nc.sync.dma_start(out=w2[f][:], in_=w2T[ts(f, 128), :])
            for k in range(KD):
                nc.sync.dma_start(out=w1[k][:], in_=w1T[ts(k, 128), :])

        # ---- projections (pj-tag PSUM, [128, 512] tiles) ----
        def qproj(t):
            for c in range(2):
                p = ps.tile([128, 512], F32, tag="pj", bufs=2, name=f"psq{t}_{c}")
                for k in range(KD):
                    nc.tensor.matmul(p[:], wq[k][:, ts(t, 128)], xt[k][:, ts(c, 512)],
                                     start=(k == 0), stop=(k == KD - 1))
                nc.vector.tensor_copy(qt[t][:, ts(c, 512)], p[:])

        def kproj(t, n):
            p = ps.tile([128, 512], F32, tag="pj", bufs=2, name=f"psk{t}_{n}")
            for k in range(KD):
                nc.tensor.matmul(p[:], wk[k][:, ts(t, 128)], yt[k][:, ts(n, 512)],
                                 start=(k == 0), stop=(k == KD - 1))
            nc.vector.tensor_copy(kt[t][:, ts(n, 512)], p[:])

        def vproj(j):
            p = ps.tile([128, 512], F32, tag="pj", bufs=2, name=f"psv{j}")
            for k in range(KD):
                nc.tensor.matmul(p[:, 0:D], yt[k][:, ts(j, 128)], wv[k][:],
                                 start=(k == 0), stop=(k == KD - 1))
            va3 = vaug[j][:].rearrange("p (h e) -> p h e", h=H)
            ps3 = p[:, 0:D].rearrange("p (h e) -> p h e", h=H)
            nc.vector.tensor_copy(va3[:, :, 0:DH], ps3[:, :, 0:DH])
            nc.vector.memset(va3[:, :, DH:DH + 1], 1.0)

        kproj(0, 0)
        kproj(0, 1)
        qproj(0)
        kproj(0, 2)
        kproj(0, 3)
        for j in range(4):
            vproj(j)
        qproj(1)
        for n in range(4):
            kproj(1, n)

        # background work emitted inside the attention j-loops: (t, j) -> fn
        bg = {}
        bg[(0, 1)] = [lambda: vproj(4), lambda: vproj(5)]
        bg[(0, 3)] = [lambda: vproj(6), lambda: vproj(7)]
        bg[(0, 5)] = [lambda: vproj(8), lambda: vproj(9)]
        bg[(0, 7)] = [lambda: vproj(10), lambda: vproj(11)]
        bg[(0, 9)] = [lambda: vproj(12), lambda: vproj(13)]
        bg[(0, 11)] = [lambda: vproj(14), lambda: vproj(15)]
        bg[(0, 13)] = [lambda: qproj(2)]
        bg[(1, 1)] = [lambda: kproj(2, 0), lambda: kproj(2, 1)]
        bg[(1, 5)] = [lambda: kproj(2, 2), lambda: kproj(2, 3)]
        bg[(1, 9)] = [lambda: qproj(3)]
        bg[(1, 13)] = [lambda: kproj(3, 0), lambda: kproj(3, 1)]
        bg[(2, 1)] = [lambda: kproj(3, 2), lambda: kproj(3, 3)]
        bg[(2, 5)] = [load_ffn_weights]

        # ---- attention ----
        def normalize_head(t, a, acc, i):
            """o_r cols for head a of pair t, query i-tile, from acc[:, 64i:...]."""
            rc = sb.tile([128, 1], F32, tag="rc", bufs=8, name=f"rc{t}_{a}_{i}")
            nc.vector.reciprocal(rc[:], acc[:, PH * i + DH:PH * i + KH])
            nc.vector.tensor_scalar(
                o_r[(t, i)][:, PH * a:PH * a + KH], acc[:, PH * i:PH * i + KH],
                rc[:], None, ALU.mult)

        o_r = {}
        pending = None

        def drain_pending():
            # normalize + transpose-back + residual + compaction for pair t
            t, accA, accB = pending
            for i in range(NI):
                o_r[(t, i)] = sb.tile([128, 128], F16, tag="or", bufs=6,
                                      name=f"or{t}_{i}")
                normalize_head(t, 0, accA, i)
                normalize_head(t, 1, accB, i)
            for i in range(NI):
                tp = ps.tile([128, 512], F32, tag="pj", bufs=2, name=f"tp{t}_{i}")
                tpv = tp[:, 0:64].bitcast(F16)
                nc.tensor.transpose(tpv, o_r[(t, i)][:], ident[:])
                nc.vector.scalar_tensor_tensor(
                    ot_p[t][:, ts(i, 128)], tpv, 1.0, qt[t][:, ts(i, 128)],
                    ALU.mult, ALU.add)
            for st_, sr, dt_, dr, nr in CSEGS:
                if st_ == t:
                    nc.sync.dma_start(out=otc[dt_][dr:dr + nr, :],
                                      in_=ot_p[t][sr:sr + nr, :])

        for t in range(TQ):
            accA = ps.tile([128, 512], F32, tag="acc", bufs=2, name=f"accA{t}")
            accB = ps.tile([128, 512], F32, tag="acc", bufs=2, name=f"accB{t}")
            nc.vector.memset(accA[:], 0.0)
            nc.vector.memset(accB[:], 0.0)
            for j in range(NJ):
                st2 = []
                for a in range(2):
                    stx = ps.tile([128, 1024], F32, tag="st", bufs=2,
                                  name=f"st{t}_{j}_{a}")
                    for c in range(2):
                        nc.tensor.matmul(
                            stx[:, ts(c, 512)],
                            kt[t][PH * a:PH * a + DH, ts(j, 128)],
                            qt[t][PH * a:PH * a + DH, ts(c, 512)],
                            start=True, stop=True)
                    st2.append(stx)
                pe2 = []
                for a in range(2):
                    pe = sb.tile([128, 1024], F16, tag="pt", bufs=6,
                                 name=f"pe{t}_{j}_{a}")
                    if (a, j) in DVE_EXP:
                        nc.vector.tensor_scalar(
                            pe[:].bitcast(I16), st2[a][:], A16, B16,
                            ALU.mult, ALU.add)
                    else:
                        nc.scalar.activation(pe[:], st2[a][:], AF.Exp, scale=SCALE)
                    pe2.append(pe)
                for fn in bg.get((t, j), ()):
                    fn()
                if j == 1 and pending is not None:
                    drain_pending()
                for a, acc in ((0, accA), (1, accB)):
                    for i in range(NI):
                        nc.tensor.matmul(
                            acc[:, PH * i:PH * i + KH],
                            pe2[a][:, ts(i, 128)],
                            vaug[j][:, PH * (2 * t + a):PH * (2 * t + a) + KH],
                            start=False, stop=(j == NJ - 1),
                            skip_group_check=True)
            pending = (t, accA, accB)
        drain_pending()

        # ---- FFN (feature-major, compact): otc -> gelu(W1@otc) -> W2@hid + otc
        for c in range(2):
            hf = []
            for g in range(NF // 2):
                sg = ps.tile([128, 1024], F32, tag="st", bufs=2, name=f"sg{c}_{g}")
                for fi in range(2):
                    for k in range(KD):
                        nc.tensor.matmul(
                            sg[:, ts(fi, 512)],
                            w1[k][:, ts(g * 2 + fi, 128)], otc[k][:, ts(c, 512)],
                            start=(k == 0), stop=(k == KD - 1))
                h = sb.tile([128, 1024], F16, tag="hid", bufs=8, name=f"hf{c}_{g}")
                nc.scalar.activation(h[:], sg[:], AF.Gelu)
                hf.append(h)
            for m in range(KD):
                po = ps.tile([128, 512], F32, tag="acc", bufs=2, name=f"po{c}_{m}")
                for g in range(NF // 2):
                    for fi in range(2):
                        nc.tensor.matmul(
                            po[:], w2[g * 2 + fi][:, ts(m, 128)],
                            hf[g][:, ts(fi, 512)],
                            start=(g == 0 and fi == 0),
                            stop=(g == NF // 2 - 1 and fi == 1))
                osb = sb.tile([128, 512], F32, tag="osb", bufs=3, name=f"osb{c}_{m}")
                nc.vector.tensor_add(osb[:], po[:], otc[m][:, ts(c, 512)])
                nc.sync.dma_start(out=o[ts(m, 128), ts(c, 512)], in_=osb[:])

    nc.compile()
    return nc


def _prep_weights(Wq, Wk, Wv, W1, W2):
    def pad_rows(w):  # [384, X] -> [512, X]; head h dims at rows 64h..64h+47
        out = np.zeros((DP,) + w.shape[1:], dtype=w.dtype)
        out.reshape(H, PH, -1)[:, 0:DH] = w.reshape(H, DH, -1)
        return out

    f16 = np.float16
    wqT = np.ascontiguousarray(pad_rows(Wq).T).astype(f16)    # [384, 512]
    wkT = np.ascontiguousarray(pad_rows(Wk).T).astype(f16)    # [384, 512]
    wvT = np.ascontiguousarray(Wv.T).astype(f16)              # [384, 384]
    w1T = np.ascontiguousarray(W1.T).astype(f16)              # [384, 1536]
    w2T = np.ascontiguousarray(W2.T).astype(f16)              # [1536, 384]
    return wqT, wkT, wvT, w1T, w2T


def _run(in_maps, trace=False):
    from concourse.bass_utils import run_bass_kernel_spmd

    if "nc" not in _CACHE:
        _CACHE["nc"] = _build()
    try:
        return run_bass_kernel_spmd(_CACHE["nc"], in_maps, list(range(8)), trace=trace)
    except Exception:
        # one retry: absorbs transient device wedges (NRT_EXEC_UNIT_* from a
        # previous interrupted run on the shared tunneled devices). Once PJRT
        # marks a device unrecoverable the client is poisoned, so drop the
        # cached backends to force a fresh client before retrying.
        import time as _time
        last = None
        for delay in (10.0, 30.0):
            try:
                import jax
                import jax._src.xla_bridge as _xb
                jax.clear_caches()
                with _xb._backend_lock:
                    _xb._backends.clear()
                    _xb._backend_errors.clear()
            except Exception:
                pass
            _time.sleep(delay)
            try:
                return run_bass_kernel_spmd(_CACHE["nc"], in_maps,
                                            list(range(8)), trace=trace)
            except Exception as e:  # noqa
                last = e
        raise last


def _make_in_maps(x, y, Wq, Wk, Wv, W1, W2):
    x = np.asarray(x, dtype=np.float32)
    y = np.asarray(y, dtype=np.float32)
    wqT, wkT, wvT, w1T, w2T = _prep_weights(
        np.asarray(Wq, np.float32), np.asarray(Wk, np.float32),
        np.asarray(Wv, np.float32), np.asarray(W1, np.float32),
        np.asarray(W2, np.float32))
    ident = np.eye(128, dtype=np.float16)
    in_maps = []
    for c in range(8):
        b, half = c // 2, c % 2
        xs = x[b, half * ROWS:(half + 1) * ROWS]  # [1024, 384]
        in_maps.append({
            "xT": np.ascontiguousarray(xs.T).astype(np.float16),
            "yT": np.ascontiguousarray(y[b].T).astype(np.float16),
            "wqT": wqT, "wkT": wkT, "wvT": wvT, "w1T": w1T, "w2T": w2T,
            "idT": ident,
        })
    return in_maps


def _unshard(results):
    out = np.empty((B, N, D), np.float32)
    for c in range(8):
        oc = results[c]["o"]  # [384, 1024] compact feature-major
        out[c // 2, (c % 2) * ROWS:(c % 2 + 1) * ROWS, :] = oc.T
    return out


def kernel(x, y, Wq, Wk, Wv, W1, W2):
    res = _run(_make_in_maps(x, y, Wq, Wk, Wv, W1, W2))
    return _unshard(res.results)


def profile(x, y, Wq, Wk, Wv, W1, W2):
    """Run with NTFF tracing; returns exec_time_ns (or None)."""
    import concourse.bass_utils as bu
    orig = bu.upload_artifacts
    bu.upload_artifacts = lambda tmpdir: f"file://{tmpdir}"
    try:
        res = _run(_make_in_maps(x, y, Wq, Wk, Wv, W1, W2), trace=True)
    finally:
        bu.upload_artifacts = orig
    return res.exec_time_ns


# revision 9
# speedup vs baseline: 1.2024x; 1.0480x over previous
"""Multi-head self-attention block (B=4, N=2048, D=384, H=8, FF=1536) on 8 TRN2 cores.

Sharding: data-parallel over tokens. Core c handles batch b=c//2, query rows
[(c%2)*1024, (c%2+1)*1024). K/V are computed per-batch on each core (2x
replicated work, zero collectives). Everything runs fp16 on the PE inputs
(f32 PSUM accumulation); the host pre-casts/pads inputs and unpads the output.

Head padding: each 48-dim head occupies a 64-slot block:
  slots 0-47 = head dims, slot 48 = softmax-denominator slot, 49-63 = junk.
Q/K are feature-major [512pad, n] with that row layout (wq/wk host-padded with
zero rows so the pad rows are zero). V is row-major "augmented": vaug[j] =
[128 keys, 8*64] with per-head block cols [V dims 0-47 | 1.0 | junk]; the ones
column makes the P@V matmul drop the softmax denominator into output col 48.

Attention datapath per head pair t (heads 2t, 2t+1):
  scores  S[j-tile, q] = K^T Q     (PSUM f32, keys on partitions)
  exp     ACT Exp for most tiles; a subset runs on DVE via the Schraudolph
          bit-trick (out_i16 = s*A16 + B16, bitcast to fp16) to offload the
          ACT engine, which is otherwise the bottleneck.
  P@V     TRANSPOSED: out[q, v] = sum_j P[j,q] V[j,v] -- queries on output
          partitions (full 128-wide PE use; 49-wide moving dim). 8 i-tile
          accumulators per head packed at 64-col offsets into one PSUM bank,
          zeroed by DVE memset and accumulated with start=False matmuls.
  norm    denominator is per-partition (col 48) -> DVE reciprocal +
          tensor_scalar multiply into o_r [128 q, 128] fp16 (A cols 0-48,
          B cols 64-112).
  back    one PE transpose per (t, i) -> [128 v, 128 q] fp16 in PSUM, then
          one DVE scalar_tensor_tensor adds the Q residual while copying to
          the padded feature-major ot_p.
ot_p is compacted 512->384 rows by 10 partition-moving SBUF->SBUF DMAs, then
the FFN (fp16 weights, f32 PSUM) runs over compact dims with a fused final
residual add.

PSUM budget (8 banks): st 2x[128,1024] (4) + acc 2x[128,512] (2) +
pj 2x[128,512] (2). pj serves projections, transposes (bitcast fp16 view),
and is free for FFN; acc serves attention accumulators and FFN2 accumulators.
"""

import math
import numpy as np

B, N, D, H, DH, DFF = 4, 2048, 384, 8, 48, 1536
PH = 64            # padded per-head block
DP = H * PH        # 512 padded model dim
ROWS = 1024        # query rows per core
KD = D // 128      # 3 k-tiles over model dim
TQ = DP // 128     # 4 tiles over padded dim (= head pairs)
NJ = N // 128      # 16 key tiles
NI = ROWS // 128   # 8 query i-tiles
NF = DFF // 128    # 12 ffn tiles
KH = DH + 1        # 49 cols per head block incl denominator col
SCALE = 1.0 / math.sqrt(D)

# Schraudolph fp16 exp: bitcast_f16(int16(s*A16 + B16)) ~= exp(s*SCALE)
A16 = SCALE * 1024.0 / math.log(2.0)
B16 = 15.0 * 1024.0 - 60.0

# exp tiles routed to DVE instead of ACT: (head_in_pair, j) pairs
DVE_EXP = {(1, j) for j in range(NJ)}


# DMA segments to compact padded ot_p [512 rows] -> otc [384 rows]:
# (src_tile, src_row, dst_tile, dst_row, nrows)
def _compact_segs():
    segs = []
    for h in range(H):
        s_lo, d, left, off = 64 * (h % 2), DH * h, DH, 0
        while left:
            n = min(left, 128 - ((d + off) % 128))
            segs.append((h // 2, s_lo + off, (d + off) // 128, (d + off) % 128, n))
            off += n
            left -= n
    return segs


CSEGS = _compact_segs()

_CACHE = {}


def _build():
    from contextlib import ExitStack
    import concourse.bass as bass
    import concourse.bacc as bacc
    import concourse.tile as tile
    import concourse.mybir as mybir

    F32 = mybir.dt.float32
    F16 = mybir.dt.float16
    I16 = mybir.dt.int16
    AF = mybir.ActivationFunctionType
    ALU = mybir.AluOpType
    ts = bass.ts

    nc = bacc.Bacc(trn_type="TRN2", target_bir_lowering=False, debug=False)

    def din(name, shape, dt=F16):
        return nc.dram_tensor(name, shape, dt, kind="ExternalInput").ap()

    xT = din("xT", [D, ROWS])
    yT = din("yT", [D, N])
    wqT = din("wqT", [D, DP])
    wkT = din("wkT", [D, DP])
    wvT = din("wvT", [D, D])
    w1T = din("w1T", [D, DFF])
    w2T = din("w2T", [DFF, D])
    idT = din("idT", [128, 128])
    o = nc.dram_tensor("o", [D, ROWS], F32, kind="ExternalOutput").ap()

    with tile.TileContext(nc) as tc, ExitStack() as ctx:
        sb = ctx.enter_context(tc.tile_pool(name="sb", bufs=1))
        ps = ctx.enter_context(tc.tile_pool(name="ps", bufs=1, space="PSUM"))

        # ---- persistent SBUF tiles ----
        xt = [sb.tile([128, ROWS], F16, tag="xt", bufs=3, name=f"xt{k}") for k in range(KD)]
        yt = [sb.tile([128, N], F16, tag="yt", bufs=3, name=f"yt{k}") for k in range(KD)]
        wq = [sb.tile([128, DP], F16, tag="wqk", bufs=6, name=f"wq{k}") for k in range(KD)]
        wk = [sb.tile([128, DP], F16, tag="wqk", bufs=6, name=f"wk{k}") for k in range(KD)]
        wv = [sb.tile([128, D], F16, tag="wv", bufs=3, name=f"wv{k}") for k in range(KD)]
        qt = [sb.tile([128, ROWS], F16, tag="qt", bufs=4, name=f"qt{t}") for t in range(TQ)]
        kt = [sb.tile([128, N], F16, tag="kt", bufs=4, name=f"kt{t}") for t in range(TQ)]
        vaug = [sb.tile([128, DP], F16, tag="va", bufs=16, name=f"va{j}") for j in range(NJ)]
        ident = sb.tile([128, 128], F16, tag="id", bufs=1, name="ident")
        ot_p = [sb.tile([128, ROWS], F16, tag="otp", bufs=4, name=f"otp{t}") for t in range(TQ)]
        otc = [sb.tile([128, ROWS], F16, tag="otc", bufs=3, name=f"otc{m}") for m in range(KD)]
        w1 = [sb.tile([128, DFF], F16, tag="w1", bufs=3, name=f"w1_{k}") for k in range(KD)]
        w2 = [sb.tile([128, D], F16, tag="w2", bufs=12, name=f"w2_{f}") for f in range(NF)]

        # ---- input loads (critical-path order: kproj(0,0) needs wk+yt[:,0:512],
        # then qproj needs wq+xt; bulk loads go via the idle gpsimd sequencer)
        for k in range(KD):
            nc.sync.dma_start(out=wk[k][:], in_=wkT[ts(k, 128), :])
            nc.sync.dma_start(out=yt[k][:, 0:512], in_=yT[ts(k, 128), 0:512])
        for k in range(KD):
            nc.sync.dma_start(out=wq[k][:], in_=wqT[ts(k, 128), :])
            nc.sync.dma_start(out=xt[k][:], in_=xT[ts(k, 128), :])
        for k in range(KD):
            nc.sync.dma_start(out=yt[k][:, 512:1024], in_=yT[ts(k, 128), 512:1024])
            nc.sync.dma_start(out=wv[k][:], in_=wvT[ts(k, 128), :])
        nc.sync.dma_start(out=ident[:], in_=idT[:, :])
        for k in range(KD):
            nc.gpsimd.dma_start(out=yt[k][:, 1024:2048], in_=yT[ts(k, 128), 1024:2048])

        def load_ffn_weights():
            for f in range(NF):
                nc.gpsimd.dma_start(out=w2[f][:], in_=w2T[ts(f, 128), :])
            for k in range(KD):
                nc.gpsimd.dma_start(out=w1[k][:], in_=w1T[ts(k, 128), :])

        # ---- projections (pj-tag PSUM, [128, 512] tiles) ----
        def qproj(t):
            for c in range(2):
                p = ps.tile([128, 512], F32, tag="pj", bufs=2, name=f"psq{t}_{c}")
                for k in range(KD):
                    nc.tensor.matmul(p[:], wq[k][:, ts(t, 128)], xt[k][:, ts(c, 512)],
                                     start=(k == 0), stop=(k == KD - 1))
                nc.gpsimd.tensor_copy(qt[t][:, ts(c, 512)], p[:])

        def kproj(t, n):
            p = ps.tile([128, 512], F32, tag="pj", bufs=2, name=f"psk{t}_{n}")
            for k in range(KD):
                nc.tensor.matmul(p[:], wk[k][:, ts(t, 128)], yt[k][:, ts(n, 512)],
                                 start=(k == 0), stop=(k == KD - 1))
            nc.gpsimd.tensor_copy(kt[t][:, ts(n, 512)], p[:])

        def vproj(j):
            p = ps.tile([128, 512], F32, tag="pj", bufs=2, name=f"psv{j}")
            for k in range(KD):
                nc.tensor.matmul(p[:, 0:D], yt[k][:, ts(j, 128)], wv[k][:],
                                 start=(k == 0), stop=(k == KD - 1))
            va3 = vaug[j][:].rearrange("p (h e) -> p h e", h=H)
            ps3 = p[:, 0:D].rearrange("p (h e) -> p h e", h=H)
            nc.gpsimd.tensor_copy(va3[:, :, 0:DH], ps3[:, :, 0:DH])
            nc.gpsimd.memset(va3[:, :, DH:DH + 1], 1.0)

        kproj(0, 0)
        kproj(0, 1)
        qproj(0)
        kproj(0, 2)
        kproj(0, 3)
        for j in range(4):
            vproj(j)
        qproj(1)
        for n in range(4):
            kproj(1, n)

        # background work emitted inside the attention j-loops: (t, j) -> fn
        bg = {}
        bg[(0, 1)] = [lambda: vproj(4), lambda: vproj(5)]
        bg[(0, 3)] = [lambda: vproj(6), lambda: vproj(7)]
        bg[(0, 5)] = [lambda: vproj(8), lambda: vproj(9)]
        bg[(0, 7)] = [lambda: vproj(10), lambda: vproj(11)]
        bg[(0, 9)] = [lambda: vproj(12), lambda: vproj(13)]
        bg[(0, 11)] = [lambda: vproj(14), lambda: vproj(15)]
        bg[(0, 13)] = [lambda: qproj(2)]
        bg[(1, 1)] = [lambda: kproj(2, 0), lambda: kproj(2, 1)]
        bg[(1, 5)] = [lambda: kproj(2, 2), lambda: kproj(2, 3)]
        bg[(1, 9)] = [lambda: qproj(3)]
        bg[(1, 13)] = [lambda: kproj(3, 0), lambda: kproj(3, 1)]
        bg[(2, 1)] = [lambda: kproj(3, 2), lambda: kproj(3, 3)]
        bg[(2, 5)] = [load_ffn_weights]

        # ---- attention ----
        def normalize_head(t, a, acc, i):
            """o_r cols for head a of pair t, query i-tile, from acc[:, 64i:...]."""
            rc = sb.tile([128, 1], F32, tag="rc", bufs=8, name=f"rc{t}_{a}_{i}")
            nc.vector.reciprocal(rc[:], acc[:, PH * i + DH:PH * i + KH])
            nc.gpsimd.tensor_scalar(
                o_r[(t, i)][:, PH * a:PH * a + KH], acc[:, PH * i:PH * i + KH],
                rc[:], None, ALU.mult)

        o_r = {}
        pending = None

        def drain_pending():
            # normalize + transpose-back + residual + compaction for pair t
            t, accA, accB = pending
            for i in range(NI):
                o_r[(t, i)] = sb.tile([128, 128], F16, tag="or", bufs=6,
                                      name=f"or{t}_{i}")
                normalize_head(t, 0, accA, i)
                normalize_head(t, 1, accB, i)
            for i in range(NI):
                tp = ps.tile([128, 512], F32, tag="pj", bufs=2, name=f"tp{t}_{i}")
                tpv = tp[:, 0:64].bitcast(F16)
                nc.tensor.transpose(tpv, o_r[(t, i)][:], ident[:])
                nc.vector.scalar_tensor_tensor(
                    ot_p[t][:, ts(i, 128)], tpv, 1.0, qt[t][:, ts(i, 128)],
                    ALU.mult, ALU.add)
            for st_, sr, dt_, dr, nr in CSEGS:
                if st_ == t:
                    nc.gpsimd.dma_start(out=otc[dt_][dr:dr + nr, :],
                                        in_=ot_p[t][sr:sr + nr, :])

        for t in range(TQ):
            accA = ps.tile([128, 512], F32, tag="acc", bufs=2, name=f"accA{t}")
            accB = ps.tile([128, 512], F32, tag="acc", bufs=2, name=f"accB{t}")
            nc.gpsimd.memset(accA[:], 0.0)
            nc.gpsimd.memset(accB[:], 0.0)
            for j in range(NJ):
                st2 = []
                for a in range(2):
                    stx = ps.tile([128, 1024], F32, tag="st", bufs=2,
                                  name=f"st{t}_{j}_{a}")
                    for c in range(2):
                        nc.tensor.matmul(
                            stx[:, ts(c, 512)],
                            kt[t][PH * a:PH * a + DH, ts(j, 128)],
                            qt[t][PH * a:PH * a + DH, ts(c, 512)],
                            start=True, stop=True)
                    st2.append(stx)
                pe2 = []
                for a in range(2):
                    pe = sb.tile([128, 1024], F16, tag="pt", bufs=6,
                                 name=f"pe{t}_{j}_{a}")
                    if (a, j) in DVE_EXP:
                        nc.vector.tensor_scalar(
                            pe[:].bitcast(I16), st2[a][:], A16, B16,
                            ALU.mult, ALU.add)
                    else:
                        nc.scalar.activation(pe[:], st2[a][:], AF.Exp, scale=SCALE)
                    pe2.append(pe)
                for fn in bg.get((t, j), ()):
                    fn()
                if j == 1 and pending is not None:
                    drain_pending()
                for a, acc in ((0, accA), (1, accB)):
                    for i in range(NI):
                        nc.tensor.matmul(
                            acc[:, PH * i:PH * i + KH],
                            pe2[a][:, ts(i, 128)],
                            vaug[j][:, PH * (2 * t + a):PH * (2 * t + a) + KH],
                            start=False, stop=(j == NJ - 1),
                            skip_group_check=True)
            pending = (t, accA, accB)
        drain_pending()

        # ---- FFN (feature-major, compact): otc -> gelu(W1@otc) -> W2@hid + otc
        for c in range(2):
            hf = []
            for g in range(NF // 2):
                sg = ps.tile([128, 1024], F32, tag="st", bufs=2, name=f"sg{c}_{g}")
                for fi in range(2):
                    for k in range(KD):
                        nc.tensor.matmul(
                            sg[:, ts(fi, 512)],
                            w1[k][:, ts(g * 2 + fi, 128)], otc[k][:, ts(c, 512)],
                            start=(k == 0), stop=(k == KD - 1))
                h = sb.tile([128, 1024], F16, tag="hid", bufs=8, name=f"hf{c}_{g}")
                nc.scalar.activation(h[:], sg[:], AF.Gelu)
                hf.append(h)
            for m in range(KD):
                po = ps.tile([128, 512], F32, tag="acc", bufs=2, name=f"po{c}_{m}")
                for g in range(NF // 2):
                    for fi in range(2):
                        nc.tensor.matmul(
                            po[:], w2[g * 2 + fi][:, ts(m, 128)],
                            hf[g][:, ts(fi, 512)],
                            start=(g == 0 and fi == 0),
                            stop=(g == NF // 2 - 1 and fi == 1))
                osb = sb.tile([128, 512], F32, tag="osb", bufs=3, name=f"osb{c}_{m}")
                nc.gpsimd.tensor_add(osb[:], po[:], otc[m][:, ts(c, 512)])
                nc.sync.dma_start(out=o[ts(m, 128), ts(c, 512)], in_=osb[:])

    nc.compile()
    return nc


def _prep_weights(Wq, Wk, Wv, W1, W2):
    def pad_rows(w):  # [384, X] -> [512, X]; head h dims at rows 64h..64h+47
        out = np.zeros((DP,) + w.shape[1:], dtype=w.dtype)
        out.reshape(H, PH, -1)[:, 0:DH] = w.reshape(H, DH, -1)
        return out

    f16 = np.float16
    wqT = np.ascontiguousarray(pad_rows(Wq).T).astype(f16)    # [384, 512]
    wkT = np.ascontiguousarray(pad_rows(Wk).T).astype(f16)    # [384, 512]
    wvT = np.ascontiguousarray(Wv.T).astype(f16)              # [384, 384]
    w1T = np.ascontiguousarray(W1.T).astype(f16)              # [384, 1536]
    w2T = np.ascontiguousarray(W2.T).astype(f16)              # [1536, 384]
    return wqT, wkT, wvT, w1T, w2T


def _run(in_maps, trace=False):
    from concourse.bass_utils import run_bass_kernel_spmd

    if "nc" not in _CACHE:
        _CACHE["nc"] = _build()
    try:
        return run_bass_kernel_spmd(_CACHE["nc"], in_maps, list(range(8)), trace=trace)
    except Exception:
        # one retry: absorbs transient device wedges (NRT_EXEC_UNIT_* from a
        # previous interrupted run on the shared tunneled devices). Once PJRT
        # marks a device unrecoverable the client is poisoned, so drop the
        # cached backends to force a fresh client before retrying.
        import time as _time
        last = None
        for delay in (10.0, 30.0):
            try:
                import jax
                import jax._src.xla_bridge as _xb
                jax.clear_caches()
                with _xb._backend_lock:
                    _xb._backends.clear()
                    _xb._backend_errors.clear()
            except Exception:
                pass
            _time.sleep(delay)
            try:
                return run_bass_kernel_spmd(_CACHE["nc"], in_maps,
                                            list(range(8)), trace=trace)
            except Exception as e:  # noqa
                last = e
        raise last


def _make_in_maps(x, y, Wq, Wk, Wv, W1, W2):
    x = np.asarray(x, dtype=np.float32)
    y = np.asarray(y, dtype=np.float32)
    wqT, wkT, wvT, w1T, w2T = _prep_weights(
        np.asarray(Wq, np.float32), np.asarray(Wk, np.float32),
        np.asarray(Wv, np.float32), np.asarray(W1, np.float32),
        np.asarray(W2, np.float32))
    ident = np.eye(128, dtype=np.float16)
    in_maps = []
    for c in range(8):
        b, half = c // 2, c % 2
        xs = x[b, half * ROWS:(half + 1) * ROWS]  # [1024, 384]
        in_maps.append({
            "xT": np.ascontiguousarray(xs.T).astype(np.float16),
            "yT": np.ascontiguousarray(y[b].T).astype(np.float16),
            "wqT": wqT, "wkT": wkT, "wvT": wvT, "w1T": w1T, "w2T": w2T,
            "idT": ident,
        })
    return in_maps


def _unshard(results):
    out = np.empty((B, N, D), np.float32)
    for c in range(8):
        oc = results[c]["o"]  # [384, 1024] compact feature-major
        out[c // 2, (c % 2) * ROWS:(c % 2 + 1) * ROWS, :] = oc.T
    return out


def kernel(x, y, Wq, Wk, Wv, W1, W2):
    res = _run(_make_in_maps(x, y, Wq, Wk, Wv, W1, W2))
    return _unshard(res.results)


def profile(x, y, Wq, Wk, Wv, W1, W2):
    """Run with NTFF tracing; returns exec_time_ns (or None)."""
    import concourse.bass_utils as bu
    orig = bu.upload_artifacts
    bu.upload_artifacts = lambda tmpdir: f"file://{tmpdir}"
    try:
        res = _run(_make_in_maps(x, y, Wq, Wk, Wv, W1, W2), trace=True)
    finally:
        bu.upload_artifacts = orig
    return res.exec_time_ns


# revision 15
# speedup vs baseline: 1.4586x; 1.2130x over previous
"""Multi-head self-attention block (B=4, N=2048, D=384, H=8, FF=1536) on 8 TRN2 cores.

Sharding: data-parallel over tokens. Core c handles batch b=c//2, query rows
[(c%2)*1024, (c%2+1)*1024). K/V are computed per-batch on each core (2x
replicated work, zero collectives). Everything runs fp16 on the PE inputs
(f32 PSUM accumulation); the host pre-casts/pads inputs and unpads the output.

Head padding: each 48-dim head occupies a 64-slot block:
  slots 0-47 = head dims, slot 48 = softmax-denominator slot, 49-63 = junk.
Q/K are feature-major [512pad, n] with that row layout (wq/wk host-padded with
zero rows so the pad rows are zero). V is row-major "augmented": vaug[j] =
[128 keys, 8*64] with per-head block cols [V dims 0-47 | 1.0 | junk]; the ones
column makes the P@V matmul drop the softmax denominator into output col 48.

Attention datapath per head pair t (heads 2t, 2t+1):
  scores  S[j-tile, q] = K^T Q     (PSUM f32, keys on partitions)
  exp     ACT Exp for most tiles; a subset runs on DVE via the Schraudolph
          bit-trick (out_i16 = s*A16 + B16, bitcast to fp16) to offload the
          ACT engine, which is otherwise the bottleneck.
  P@V     TRANSPOSED: out[q, v] = sum_j P[j,q] V[j,v] -- queries on output
          partitions (full 128-wide PE use; 49-wide moving dim). 8 i-tile
          accumulators per head packed at 64-col offsets into one PSUM bank,
          zeroed by DVE memset and accumulated with start=False matmuls.
  norm    denominator is per-partition (col 48) -> DVE reciprocal +
          tensor_scalar multiply into o_r [128 q, 128] fp16 (A cols 0-48,
          B cols 64-112).
  back    one PE transpose per (t, i) -> [128 v, 128 q] fp16 in PSUM, then
          one DVE scalar_tensor_tensor adds the Q residual while copying to
          the padded feature-major ot_p.
ot_p is compacted 512->384 rows by 10 partition-moving SBUF->SBUF DMAs, then
the FFN (fp16 weights, f32 PSUM) runs over compact dims with a fused final
residual add.

PSUM budget (8 banks): st 2x[128,1024] (4) + acc 2x[128,512] (2) +
pj 2x[128,512] (2). pj serves projections, transposes (bitcast fp16 view),
and is free for FFN; acc serves attention accumulators and FFN2 accumulators.
"""

import math
import numpy as np

B, N, D, H, DH, DFF = 4, 2048, 384, 8, 48, 1536
PH = 64            # padded per-head block
DP = H * PH        # 512 padded model dim
ROWS = 1024        # query rows per core
KD = D // 128      # 3 k-tiles over model dim
TQ = DP // 128     # 4 tiles over padded dim (= head pairs)
NJ = N // 128      # 16 key tiles
NI = ROWS // 128   # 8 query i-tiles
NF = DFF // 128    # 12 ffn tiles
KH = DH + 1        # 49 cols per head block incl denominator col
SCALE = 1.0 / math.sqrt(D)

# Schraudolph fp16 exp: bitcast_f16(int16(s*A16 + B16)) ~= exp(s*SCALE)
A16 = SCALE * 1024.0 / math.log(2.0)
B16 = 15.0 * 1024.0 - 60.0

# exp tiles routed to DVE instead of ACT: (head_in_pair, j) pairs
DVE_EXP = {(1, j) for j in range(NJ)}


# DMA segments to compact padded ot_p [512 rows] -> otc [384 rows]:
# (src_tile, src_row, dst_tile, dst_row, nrows)
def _compact_segs():
    segs = []
    for h in range(H):
        s_lo, d, left, off = 64 * (h % 2), DH * h, DH, 0
        while left:
            n = min(left, 128 - ((d + off) % 128))
            segs.append((h // 2, s_lo + off, (d + off) // 128, (d + off) % 128, n))
            off += n
            left -= n
    return segs


CSEGS = _compact_segs()

_CACHE = {}


def _build():
    from contextlib import ExitStack
    import concourse.bass as bass
    import concourse.bacc as bacc
    import concourse.tile as tile
    import concourse.mybir as mybir

    F32 = mybir.dt.float32
    F16 = mybir.dt.float16
    I16 = mybir.dt.int16
    AF = mybir.ActivationFunctionType
    ALU = mybir.AluOpType
    ts = bass.ts

    nc = bacc.Bacc(trn_type="TRN2", target_bir_lowering=False, debug=False)

    def din(name, shape, dt=F16):
        return nc.dram_tensor(name, shape, dt, kind="ExternalInput").ap()

    xT = din("xT", [D, ROWS])
    yT = din("yT", [D, N])
    wqT = din("wqT", [D, DP])
    wkT = din("wkT", [D, DP])
    wvT = din("wvT", [D, D])
    w1T = din("w1T", [D, DFF])
    w2T = din("w2T", [DFF, D])
    idT = din("idT", [128, 128])
    o = nc.dram_tensor("o", [D, ROWS], F32, kind="ExternalOutput").ap()

    with tile.TileContext(nc) as tc, ExitStack() as ctx:
        sb = ctx.enter_context(tc.tile_pool(name="sb", bufs=1))
        ps = ctx.enter_context(tc.tile_pool(name="ps", bufs=1, space="PSUM"))

        # ---- persistent SBUF tiles ----
        xt = [sb.tile([128, ROWS], F16, tag="xt", bufs=3, name=f"xt{k}") for k in range(KD)]
        yt = [sb.tile([128, N], F16, tag="yt", bufs=3, name=f"yt{k}") for k in range(KD)]
        wq = [sb.tile([128, DP], F16, tag="wqk", bufs=6, name=f"wq{k}") for k in range(KD)]
        wk = [sb.tile([128, DP], F16, tag="wqk", bufs=6, name=f"wk{k}") for k in range(KD)]
        wv = [sb.tile([128, D], F16, tag="wv", bufs=3, name=f"wv{k}") for k in range(KD)]
        qt = [sb.tile([128, ROWS], F16, tag="qt", bufs=4, name=f"qt{t}") for t in range(TQ)]
        kt = [sb.tile([128, N], F16, tag="kt", bufs=4, name=f"kt{t}") for t in range(TQ)]
        vaug = [sb.tile([128, DP], F16, tag="va", bufs=16, name=f"va{j}") for j in range(NJ)]
        ident = sb.tile([128, 128], F16, tag="id", bufs=1, name="ident")
        ot_p = [sb.tile([128, ROWS], F16, tag="otp", bufs=4, name=f"otp{t}") for t in range(TQ)]
        otc = [sb.tile([128, ROWS], F16, tag="otc", bufs=3, name=f"otc{m}") for m in range(KD)]
        w1 = [sb.tile([128, DFF], F16, tag="w1", bufs=3, name=f"w1_{k}") for k in range(KD)]
        w2 = [sb.tile([128, D], F16, tag="w2", bufs=12, name=f"w2_{f}") for f in range(NF)]

        # ---- input loads. Issue in parallel from four sequencers so the
        # critical kproj(0,0)/qproj(0) inputs land ASAP: SP takes wk+yt first
        # chunk, DVE takes wq+xt, ACT takes the second yt chunk + wv, gpsimd
        # takes the back half of yt.
        for k in range(KD):
            nc.sync.dma_start(out=wk[k][:], in_=wkT[ts(k, 128), :])
            nc.sync.dma_start(out=yt[k][:, 0:512], in_=yT[ts(k, 128), 0:512])
        for k in range(KD):
            nc.scalar.dma_start(out=wq[k][:], in_=wqT[ts(k, 128), :])
            nc.scalar.dma_start(out=xt[k][:], in_=xT[ts(k, 128), :])
        for k in range(KD):
            nc.gpsimd.dma_start(out=yt[k][:, 512:1024], in_=yT[ts(k, 128), 512:1024])
            nc.gpsimd.dma_start(out=wv[k][:], in_=wvT[ts(k, 128), :])
        nc.sync.dma_start(out=ident[:], in_=idT[:, :])
        for k in range(KD):
            nc.gpsimd.dma_start(out=yt[k][:, 1024:2048], in_=yT[ts(k, 128), 1024:2048])

        def load_ffn_weights():
            for f in range(NF):
                nc.sync.dma_start(out=w2[f][:], in_=w2T[ts(f, 128), :])
            for k in range(KD):
                nc.sync.dma_start(out=w1[k][:], in_=w1T[ts(k, 128), :])

        # ---- projections (pj-tag PSUM, [128, 512] tiles) ----
        def qproj(t):
            for c in range(2):
                p = ps.tile([128, 512], F32, tag="st", bufs=6, name=f"psq{t}_{c}")
                for k in range(KD):
                    nc.tensor.matmul(p[:], wq[k][:, ts(t, 128)], xt[k][:, ts(c, 512)],
                                     start=(k == 0), stop=(k == KD - 1))
                nc.gpsimd.tensor_copy(qt[t][:, ts(c, 512)], p[:])

        def kproj(t, n):
            p = ps.tile([128, 512], F32, tag="st", bufs=6, name=f"psk{t}_{n}")
            for k in range(KD):
                nc.tensor.matmul(p[:], wk[k][:, ts(t, 128)], yt[k][:, ts(n, 512)],
                                 start=(k == 0), stop=(k == KD - 1))
            nc.gpsimd.tensor_copy(kt[t][:, ts(n, 512)], p[:])

        def vproj(j):
            p = ps.tile([128, 512], F32, tag="st", bufs=6, name=f"psv{j}")
            for k in range(KD):
                nc.tensor.matmul(p[:, 0:D], yt[k][:, ts(j, 128)], wv[k][:],
                                 start=(k == 0), stop=(k == KD - 1))
            va3 = vaug[j][:].rearrange("p (h e) -> p h e", h=H)
            ps3 = p[:, 0:D].rearrange("p (h e) -> p h e", h=H)
            nc.gpsimd.tensor_copy(va3[:, :, 0:DH], ps3[:, :, 0:DH])
            nc.gpsimd.memset(va3[:, :, DH:DH + 1], 1.0)

        kproj(0, 0)
        kproj(0, 1)
        qproj(0)
        kproj(0, 2)
        kproj(0, 3)
        for j in range(4):
            vproj(j)
        qproj(1)
        for n in range(4):
            kproj(1, n)

        # background work emitted inside the attention j-loops: (t, j) -> fn
        bg = {}
        bg[(0, 1)] = [lambda: vproj(4), lambda: vproj(5)]
        bg[(0, 3)] = [lambda: vproj(6), lambda: vproj(7)]
        bg[(0, 5)] = [lambda: vproj(8), lambda: vproj(9)]
        bg[(0, 7)] = [lambda: vproj(10), lambda: vproj(11)]
        bg[(0, 9)] = [lambda: vproj(12), lambda: vproj(13)]
        bg[(0, 11)] = [lambda: vproj(14), lambda: vproj(15)]
        bg[(0, 13)] = [lambda: qproj(2)]
        bg[(1, 1)] = [lambda: kproj(2, 0), lambda: kproj(2, 1)]
        bg[(1, 5)] = [lambda: kproj(2, 2), lambda: kproj(2, 3)]
        bg[(1, 9)] = [lambda: qproj(3)]
        bg[(1, 13)] = [lambda: kproj(3, 0), lambda: kproj(3, 1)]
        bg[(2, 1)] = [lambda: kproj(3, 2), lambda: kproj(3, 3)]
        bg[(2, 5)] = [load_ffn_weights]

        # ---- attention ----
        def normalize_head(t, a, acc, i):
            """o_r cols for head a of pair t, query i-tile, from acc[:, 64i:...]."""
            rc = sb.tile([128, 1], F32, tag="rc", bufs=8, name=f"rc{t}_{a}_{i}")
            nc.vector.reciprocal(rc[:], acc[:, PH * i + DH:PH * i + KH])
            nc.gpsimd.tensor_scalar(
                o_r[(t, i)][:, PH * a:PH * a + KH], acc[:, PH * i:PH * i + KH],
                rc[:], None, ALU.mult)

        o_r = {}
        pending = None

        def drain_pending():
            # normalize + transpose-back + residual + compaction for pair t
            t, accA, accB = pending
            for i in range(NI):
                o_r[(t, i)] = sb.tile([128, 128], F16, tag="or", bufs=6,
                                      name=f"or{t}_{i}")
                normalize_head(t, 0, accA, i)
                normalize_head(t, 1, accB, i)
            for i in range(NI):
                tp = ps.tile([128, 512], F32, tag="st", bufs=6,
                             name=f"tp{t}_{i}")
                tpv = tp[:, 0:64].bitcast(F16)
                nc.tensor.transpose(tpv, o_r[(t, i)][:], ident[:])
                nc.vector.scalar_tensor_tensor(
                    ot_p[t][:, ts(i, 128)], tpv, 1.0, qt[t][:, ts(i, 128)],
                    ALU.mult, ALU.add)
            for st_, sr, dt_, dr, nr in CSEGS:
                if st_ == t:
                    nc.gpsimd.dma_start(out=otc[dt_][dr:dr + nr, :],
                                        in_=ot_p[t][sr:sr + nr, :])

        for t in range(TQ):
            accA = ps.tile([128, 512], F32, tag="acc", bufs=2, name=f"accA{t}")
            accB = ps.tile([128, 512], F32, tag="acc", bufs=2, name=f"accB{t}")
            nc.gpsimd.memset(accA[:], 0.0)
            nc.gpsimd.memset(accB[:], 0.0)
            for j in range(NJ):
                pe2 = []
                for a in range(2):
                    pe = sb.tile([128, 1024], F16, tag="pt", bufs=6,
                                 name=f"pe{t}_{j}_{a}")
                    for c in range(2):
                        stx = ps.tile([128, 512], F32, tag="st", bufs=6,
                                      name=f"st{t}_{j}_{a}_{c}")
                        nc.tensor.matmul(
                            stx[:],
                            kt[t][PH * a:PH * a + DH, ts(j, 128)],
                            qt[t][PH * a:PH * a + DH, ts(c, 512)],
                            start=True, stop=True)
                        if (a, j) in DVE_EXP:
                            nc.vector.tensor_scalar(
                                pe[:, ts(c, 512)].bitcast(I16), stx[:], A16, B16,
                                ALU.mult, ALU.add)
                        else:
                            nc.scalar.activation(pe[:, ts(c, 512)], stx[:],
                                                 AF.Exp, scale=SCALE)
                    pe2.append(pe)
                for fn in bg.get((t, j), ()):
                    fn()
                if j == 1 and pending is not None:
                    drain_pending()
                for a, acc in ((0, accA), (1, accB)):
                    for i in range(NI):
                        nc.tensor.matmul(
                            acc[:, PH * i:PH * i + KH],
                            pe2[a][:, ts(i, 128)],
                            vaug[j][:, PH * (2 * t + a):PH * (2 * t + a) + KH],
                            start=False, stop=(j == NJ - 1),
                            skip_group_check=True)
            pending = (t, accA, accB)
        drain_pending()

        # ---- FFN (feature-major, compact): otc -> gelu(W1@otc) -> W2@hid + otc
        for c in range(2):
            hf = []
            for f in range(NF):
                sg = ps.tile([128, 512], F32, tag="st", bufs=6, name=f"sg{c}_{f}")
                for k in range(KD):
                    nc.tensor.matmul(
                        sg[:], w1[k][:, ts(f, 128)], otc[k][:, ts(c, 512)],
                        start=(k == 0), stop=(k == KD - 1))
                h = sb.tile([128, 512], F16, tag="hid", bufs=14, name=f"hf{c}_{f}")
                nc.scalar.activation(h[:], sg[:], AF.Gelu)
                hf.append(h)
            for m in range(KD):
                po = ps.tile([128, 512], F32, tag="acc", bufs=2, name=f"po{c}_{m}")
                for f in range(NF):
                    nc.tensor.matmul(
                        po[:], w2[f][:, ts(m, 128)], hf[f][:],
                        start=(f == 0), stop=(f == NF - 1))
                osb = sb.tile([128, 512], F32, tag="osb", bufs=3, name=f"osb{c}_{m}")
                nc.gpsimd.tensor_add(osb[:], po[:], otc[m][:, ts(c, 512)])
                nc.sync.dma_start(out=o[ts(m, 128), ts(c, 512)], in_=osb[:])

    nc.compile()
    return nc


def _prep_weights(Wq, Wk, Wv, W1, W2):
    def pad_rows(w):  # [384, X] -> [512, X]; head h dims at rows 64h..64h+47
        out = np.zeros((DP,) + w.shape[1:], dtype=w.dtype)
        out.reshape(H, PH, -1)[:, 0:DH] = w.reshape(H, DH, -1)
        return out

    f16 = np.float16
    wqT = np.ascontiguousarray(pad_rows(Wq).T).astype(f16)    # [384, 512]
    wkT = np.ascontiguousarray(pad_rows(Wk).T).astype(f16)    # [384, 512]
    wvT = np.ascontiguousarray(Wv.T).astype(f16)              # [384, 384]
    w1T = np.ascontiguousarray(W1.T).astype(f16)              # [384, 1536]
    w2T = np.ascontiguousarray(W2.T).astype(f16)              # [1536, 384]
    return wqT, wkT, wvT, w1T, w2T


def _run(in_maps, trace=False):
    from concourse.bass_utils import run_bass_kernel_spmd

    if "nc" not in _CACHE:
        _CACHE["nc"] = _build()
    try:
        return run_bass_kernel_spmd(_CACHE["nc"], in_maps, list(range(8)), trace=trace)
    except Exception:
        # one retry: absorbs transient device wedges (NRT_EXEC_UNIT_* from a
        # previous interrupted run on the shared tunneled devices). Once PJRT
        # marks a device unrecoverable the client is poisoned, so drop the
        # cached backends to force a fresh client before retrying.
        import time as _time
        last = None
        for delay in (10.0, 30.0):
            try:
                import jax
                import jax._src.xla_bridge as _xb
                jax.clear_caches()
                with _xb._backend_lock:
                    _xb._backends.clear()
                    _xb._backend_errors.clear()
            except Exception:
                pass
            _time.sleep(delay)
            try:
                return run_bass_kernel_spmd(_CACHE["nc"], in_maps,
                                            list(range(8)), trace=trace)
            except Exception as e:  # noqa
                last = e
        raise last


def _make_in_maps(x, y, Wq, Wk, Wv, W1, W2):
    x = np.asarray(x, dtype=np.float32)
    y = np.asarray(y, dtype=np.float32)
    wqT, wkT, wvT, w1T, w2T = _prep_weights(
        np.asarray(Wq, np.float32), np.asarray(Wk, np.float32),
        np.asarray(Wv, np.float32), np.asarray(W1, np.float32),
        np.asarray(W2, np.float32))
    ident = np.eye(128, dtype=np.float16)
    in_maps = []
    for c in range(8):
        b, half = c // 2, c % 2
        xs = x[b, half * ROWS:(half + 1) * ROWS]  # [1024, 384]
        in_maps.append({
            "xT": np.ascontiguousarray(xs.T).astype(np.float16),
            "yT": np.ascontiguousarray(y[b].T).astype(np.float16),
            "wqT": wqT, "wkT": wkT, "wvT": wvT, "w1T": w1T, "w2T": w2T,
            "idT": ident,
        })
    return in_maps


def _unshard(results):
    out = np.empty((B, N, D), np.float32)
    for c in range(8):
        oc = results[c]["o"]  # [384, 1024] compact feature-major
        out[c // 2, (c % 2) * ROWS:(c % 2 + 1) * ROWS, :] = oc.T
    return out


def kernel(x, y, Wq, Wk, Wv, W1, W2):
    res = _run(_make_in_maps(x, y, Wq, Wk, Wv, W1, W2))
    return _unshard(res.results)


def profile(x, y, Wq, Wk, Wv, W1, W2):
    """Run with NTFF tracing; returns exec_time_ns (or None)."""
    import concourse.bass_utils as bu
    orig = bu.upload_artifacts
    bu.upload_artifacts = lambda tmpdir: f"file://{tmpdir}"
    try:
        res = _run(_make_in_maps(x, y, Wq, Wk, Wv, W1, W2), trace=True)
    finally:
        bu.upload_artifacts = orig
    return res.exec_time_ns
